# revision 7
# baseline (speedup 1.0000x reference)
"""Trainium2 Bass kernel for AugmentedGraphNeuralODEFunc.

Reference computation (B=4, N=512, AUG=32, ORIG=16, HID=128):
  edge_features[b,i,j] = [z_i(32), z_j(32), p_i-p_j(3), |p_i-p_j|(1),
                          ps_i-ps_j(3), |ps_i-ps_j|(1)]       (72)
  msg = MLP(72->128->128->16) per edge; agg_i = sum_j msg_ij
  d_evolving = MLP(32->128->128->16)([z_i[:16], agg_i]); static half -> 0

Algebraic restructure used on device:
  layer1 pre-act for receiver i, sender j:
    h1[:,j] = W_B^T z_j + A_i + dist_ij * v + dist_s_ij * w
  where A_i = W_A^T z_i + eb0 (diff terms fold into W_A/W_B since
  diff = p_i - p_j is linear in z), v/w are the dist rows of eW0.
  dist^2 via the Gram identity r_i + r_j - 2 p_i.p_j as ONE K=5 matmul,
  then clamp + sqrt.  Layer 3 + bias commute with the sum over j:
    agg_i = (sum_j relu(h2_ij)) @ eW2 + N*eb2.

Sharding: receivers (dim 1 of the NxN edge tensor) split across 8 cores,
64 receivers x 4 batches = 256 receiver-pairs per core; the sum over
senders is local so there is no cross-core communication.

Per (b,i) pair on device:
  DMA   : A_i row -> LZ[k] row 32 (rotating lhsT buffers)
  mm_z  : psum  = LZ^T @ ZT1[b]              K=33
  mm_vw : psum += VW128[32a:32a+2]^T @ PS[b][32a:32a+2, s]   K=2
  DVE   : h1 = relu(psum)                    (PSUM->SBUF, 2 pairs/op)
  mm2   : psum1 = eW1^T @ h1                 K=128
  ACT   : relu(psum1 + eb1) with accum_out -> S[:, pair]  (the j-sum)
Tail (256 pairs at once): agg = eW2^T S + N*eb2; update MLP 32->128->128->16.

Host-side prep is restricted to O(N)-sized layout/weight folding (zt
transpose, Gram operand rows, weight scatter-folds); all O(N^2) work and
all O(N*HID) matmuls run on device.
"""

import numpy as np

import concourse.bass as bass
import concourse.tile as tile
from concourse import bacc, mybir
from concourse.bass_utils import run_bass_kernel_spmd

ORIG = 16
AUG = 32
HID = 128
B = 4
N = 512
NCORES = 8
RECV = N // NCORES          # 64 receivers per core
PAIRS = B * RECV            # 256 (b, i) pairs per core

F32 = mybir.dt.float32
F32R = mybir.dt.float32r
AluOp = mybir.AluOpType
Act = mybir.ActivationFunctionType

_PROGRAM_CACHE = {}


def build_program(use_f32r=True):
    nc = bacc.Bacc("TRN2", target_bir_lowering=False, debug=False)

    MF = F32R if use_f32r else F32

    def din(name, shape, dt=F32):
        return nc.dram_tensor(name, shape, dt, kind="ExternalInput")

    zt1_d = din("zt1", [B, 33, N], MF)        # z[b].T (32 rows) + ones row
    zr_d = din("zr", [B, 33, RECV])       # receiver slice of zt1 (per-core)
    gl_d = din("gl", [B, 37, RECV])       # Gram lhsT rows (per-core)
    gr_d = din("gr", [B, 37, N])          # Gram rhs rows
    wb32_d = din("wb32", [32, HID])       # folded sender-side layer1 weights
    wa33_d = din("wa33", [33, HID])       # folded receiver-side weights + eb0
    vw128_d = din("vw128", [128, HID], MF)    # v at rows 32a, w at rows 32a+1
    ew1_d = din("ew1", [HID, HID], MF)
    eb1_d = din("eb1", [HID, 1])
    ew2_d = din("ew2", [HID, ORIG])
    nb2_d = din("nb2", [ORIG, 1])         # N * eb2
    uw0_d = din("uw0", [AUG, HID])
    ub0_d = din("ub0", [HID, 1])
    uw1_d = din("uw1", [HID, HID])
    ub1_d = din("ub1", [HID, 1])
    uw2_d = din("uw2", [HID, ORIG])
    ub2_d = din("ub2", [ORIG, 1])
    out_d = nc.dram_tensor("out", [ORIG, PAIRS], F32, kind="ExternalOutput")

    with tile.TileContext(nc) as tc:
        with (
            tc.tile_pool(name="const", bufs=1) as cp,
            tc.tile_pool(name="work", bufs=2) as wp,
            tc.tile_pool(name="ps0", bufs=2, space=bass.MemorySpace.PSUM) as pp0,
            tc.tile_pool(name="ps1", bufs=4, space=bass.MemorySpace.PSUM) as pp1,
        ):
            # ---------------- load constants / inputs ----------------
            ZT1 = [cp.tile([33, N], MF, name=f"zt1_{b}") for b in range(B)]
            ZR = [cp.tile([33, RECV], F32, name=f"zr_{b}") for b in range(B)]
            GL = [cp.tile([37, RECV], F32, name=f"gl_{b}") for b in range(B)]
            GR = [cp.tile([37, N], F32, name=f"gr_{b}") for b in range(B)]
            for b in range(B):
                nc.sync.dma_start(ZT1[b][:], zt1_d[b])
                nc.sync.dma_start(ZR[b][:], zr_d[b])
                nc.sync.dma_start(GL[b][:], gl_d[b])
                nc.sync.dma_start(GR[b][:], gr_d[b])

            WB32 = cp.tile([32, HID], F32, name="wb32")
            WA33 = cp.tile([33, HID], F32, name="wa33")
            VW128 = cp.tile([128, HID], MF, name="vw128")
            EW1 = cp.tile([HID, HID], MF, name="ew1")
            EB1 = cp.tile([HID, 1], F32, name="eb1")
            EW2 = cp.tile([HID, ORIG], F32, name="ew2")
            NB2 = cp.tile([ORIG, 1], F32, name="nb2")
            UW0 = cp.tile([AUG, HID], F32, name="uw0")
            UB0 = cp.tile([HID, 1], F32, name="ub0")
            UW1 = cp.tile([HID, HID], F32, name="uw1")
            UB1 = cp.tile([HID, 1], F32, name="ub1")
            UW2 = cp.tile([HID, ORIG], F32, name="uw2")
            UB2 = cp.tile([ORIG, 1], F32, name="ub2")
            for t, d in [
                (WB32, wb32_d), (WA33, wa33_d), (VW128, vw128_d), (EW1, ew1_d),
                (EB1, eb1_d), (EW2, ew2_d), (NB2, nb2_d), (UW0, uw0_d),
                (UB0, ub0_d), (UW1, uw1_d), (UB1, ub1_d), (UW2, uw2_d),
                (UB2, ub2_d),
            ]:
                nc.sync.dma_start(t[:], d[:])

            # lhsT buffers for mm_z: rows 0:32 = W_B (static), row 32 = A_i
            LZ = [cp.tile([33, HID], MF, name=f"lz_{k}") for k in range(4)]
            for k in range(4):
                nc.vector.tensor_copy(LZ[k][0:32, :], WB32[:])

            # ---------------- A rows per receiver --------------------
            # AER[b][p, :] = z_i^T W_A + eb0   (row-major, feeds LZ row 32)
            AER = [cp.tile([RECV, HID], MF, name=f"aer_{b}") for b in range(B)]
            for b in range(B):
                a_ps = pp1.tile([RECV, HID], F32, tag="psum1", name="a_ps")
                nc.tensor.matmul(a_ps[:], ZR[b][:], WA33[:], start=True, stop=True)
                nc.vector.tensor_copy(AER[b][:], a_ps[:])

            # ---------------- main loop over b -----------------------
            S = cp.tile([HID, PAIRS], F32, name="s_acc")

            for b in range(B):
                # distances: Gram -> clamp -> sqrt -> pair-staged layout
                # PS_b partition 32a+t, slot s = receiver p = 16a+s;
                # t=0 dist, t=1 dist_s
                PS_b = wp.tile([128, RECV // 4, N], MF, tag="pairstage",
                               name="pairstage")
                for half in range(2):
                    ro = 32 * half
                    g_ps = pp1.tile([RECV, N], F32, tag="psum1", name="g_ps")
                    nc.tensor.matmul(
                        g_ps[:], GL[b][ro:ro + 5, :], GR[b][ro:ro + 5, :],
                        start=True, stop=True,
                    )
                    d2 = wp.tile([RECV, N], F32, tag="d2", name="d2")
                    nc.vector.tensor_scalar(
                        out=d2[:], in0=g_ps[:],
                        scalar1=0.0, scalar2=None, op0=AluOp.max,
                    )
                    dsq = wp.tile([RECV, N], MF, tag="dsq", name="dsq")
                    nc.scalar.sqrt(dsq[:], d2[:])
                    # partition remap p=16a+s -> (32a+half, s) via 4 DMAs,
                    # each reading 16 contiguous partitions into one
                    # partition's free space
                    for a in range(4):
                        nc.sync.dma_start(
                            PS_b[32 * a + half:32 * a + half + 1, :, :],
                            dsq[16 * a:16 * a + 16, :],
                        )

                for pp_i in range(0, RECV, 2):
                    psum0 = pp0.tile([128, 2 * N], F32, tag="psum0",
                                     name="psum0")
                    for t in range(2):
                        p = pp_i + t
                        q = RECV * b + p
                        k = q % 4
                        a = p // 16
                        s = p % 16
                        nc.sync.dma_start(
                            LZ[k][32:33, :], AER[b][p:p + 1, :]
                        )
                        sl = psum0[:, N * t:N * (t + 1)]
                        nc.tensor.matmul(
                            sl, LZ[k][:], ZT1[b][:],
                            start=True, stop=False,
                        )
                        nc.tensor.matmul(
                            sl,
                            VW128[32 * a:32 * a + 2, :],
                            PS_b[32 * a:32 * a + 2, s, :],
                            start=False, stop=True,
                            tile_position=(32 * a, 0),
                        )
                    h1 = wp.tile([128, 2 * N], MF, tag="h1", name="h1")
                    nc.vector.tensor_scalar(
                        out=h1[:], in0=psum0[:],
                        scalar1=0.0, scalar2=None, op0=AluOp.max,
                    )
                    for t in range(2):
                        q = RECV * b + pp_i + t
                        psum1 = pp1.tile([HID, N], F32, tag="psum1",
                                         name="psum1")
                        nc.tensor.matmul(
                            psum1[:], EW1[:],
                            h1[:, N * t:N * (t + 1)],
                            start=True, stop=True,
                        )
                        # relu(h2 + eb1), summed over senders -> S[:, q]
                        nc.scalar.activation(
                            out=psum1[:], in_=psum1[:], func=Act.Relu,
                            bias=EB1[:], scale=1.0,
                            accum_out=S[:, q:q + 1],
                        )

            # ---------------- tail: agg + update MLP -----------------
            U = cp.tile([AUG, PAIRS], F32, name="u_in")
            for b in range(B):
                nc.vector.tensor_copy(
                    U[0:ORIG, RECV * b:RECV * (b + 1)], ZR[b][0:ORIG, :]
                )
            agg_ps = pp1.tile([ORIG, PAIRS], F32, tag="psum1", name="agg_ps")
            nc.tensor.matmul(agg_ps[:], EW2[:], S[:], start=True, stop=True)
            AGGSB = cp.tile([ORIG, PAIRS], F32, name="aggsb")
            nc.vector.tensor_scalar(
                out=AGGSB[:], in0=agg_ps[:],
                scalar1=NB2[:], scalar2=None, op0=AluOp.add,
            )
            nc.sync.dma_start(U[ORIG:AUG, :], AGGSB[:])

            u1_ps = pp1.tile([HID, PAIRS], F32, tag="psum1", name="u1_ps")
            nc.tensor.matmul(u1_ps[:], UW0[:], U[:], start=True, stop=True)
            HU1 = cp.tile([HID, PAIRS], F32, name="hu1")
            nc.scalar.activation(
                out=HU1[:], in_=u1_ps[:], func=Act.Relu, bias=UB0[:], scale=1.0
            )
            u2_ps = pp1.tile([HID, PAIRS], F32, tag="psum1", name="u2_ps")
            nc.tensor.matmul(u2_ps[:], UW1[:], HU1[:], start=True, stop=True)
            HU2 = cp.tile([HID, PAIRS], F32, name="hu2")
            nc.scalar.activation(
                out=HU2[:], in_=u2_ps[:], func=Act.Relu, bias=UB1[:], scale=1.0
            )
            u3_ps = pp1.tile([ORIG, PAIRS], F32, tag="psum1", name="u3_ps")
            nc.tensor.matmul(u3_ps[:], UW2[:], HU2[:], start=True, stop=True)
            OUTSB = cp.tile([ORIG, PAIRS], F32, name="outsb")
            nc.vector.tensor_scalar(
                out=OUTSB[:], in0=u3_ps[:],
                scalar1=UB2[:], scalar2=None, op0=AluOp.add,
            )
            nc.sync.dma_start(out_d[:], OUTSB[:])

    nc.compile()
    return nc


def _host_prep(z_aug, eW0, eb0, eW1, eb1, eW2, eb2,
               uW0, ub0, uW1, ub1, uW2, ub2):
    f = np.float32
    z = np.ascontiguousarray(z_aug, dtype=f)
    zt = z.transpose(0, 2, 1)                            # [B, 32, N]
    zt1 = np.concatenate([zt, np.ones((B, 1, N), f)], axis=1)  # [B, 33, N]

    # Gram rhs rows (same for all cores):
    # rows 0-2 p_j, 3 ones, 4 r_j;  rows 32-34 ps_j, 35 ones, 36 rs_j
    gr = np.zeros((B, 37, N), f)
    p = zt[:, 0:3, :]
    ps = zt[:, 16:19, :]
    r = (p * p).sum(axis=1)
    rs = (ps * ps).sum(axis=1)
    gr[:, 0:3] = p
    gr[:, 3] = 1.0
    gr[:, 4] = r
    gr[:, 32:35] = ps
    gr[:, 35] = 1.0
    gr[:, 36] = rs

    eW0 = np.asarray(eW0, f)
    WA = eW0[0:32].copy()
    WA[0:3] += eW0[64:67]
    WA[16:19] += eW0[68:71]
    wa33 = np.concatenate([WA, np.asarray(eb0, f)[None, :]], axis=0)
    WB = eW0[32:64].copy()
    WB[0:3] -= eW0[64:67]
    WB[16:19] -= eW0[68:71]
    vw128 = np.zeros((128, HID), f)
    vw128[[0, 32, 64, 96], :] = eW0[67]
    vw128[[1, 33, 65, 97], :] = eW0[71]

    common = {
        "zt1": np.ascontiguousarray(zt1),
        "gr": gr,
        "wb32": np.ascontiguousarray(WB),
        "wa33": np.ascontiguousarray(wa33),
        "vw128": vw128,
        "ew1": np.ascontiguousarray(np.asarray(eW1, f)),
        "eb1": np.asarray(eb1, f).reshape(HID, 1).copy(),
        "ew2": np.ascontiguousarray(np.asarray(eW2, f)),
        "nb2": (np.asarray(eb2, f) * np.float32(N)).reshape(ORIG, 1).copy(),
        "uw0": np.ascontiguousarray(np.asarray(uW0, f)),
        "ub0": np.asarray(ub0, f).reshape(HID, 1).copy(),
        "uw1": np.ascontiguousarray(np.asarray(uW1, f)),
        "ub1": np.asarray(ub1, f).reshape(HID, 1).copy(),
        "uw2": np.ascontiguousarray(np.asarray(uW2, f)),
        "ub2": np.asarray(ub2, f).reshape(ORIG, 1).copy(),
    }
    in_maps = []
    for c in range(NCORES):
        sl = slice(RECV * c, RECV * (c + 1))
        m = dict(common)
        m["zr"] = np.ascontiguousarray(zt1[:, :, sl])
        gl = np.zeros((B, 37, RECV), f)
        gl[:, 0:3] = -2.0 * p[:, :, sl]
        gl[:, 3] = r[:, sl]
        gl[:, 4] = 1.0
        gl[:, 32:35] = -2.0 * ps[:, :, sl]
        gl[:, 35] = rs[:, sl]
        gl[:, 36] = 1.0
        m["gl"] = gl
        in_maps.append(m)
    return in_maps


def _assemble(results, dtype):
    out = np.zeros((B, N, AUG), dtype=dtype)
    for c in range(NCORES):
        o = results[c]["out"]                 # [ORIG, PAIRS]
        for b in range(B):
            out[b, RECV * c:RECV * (c + 1), 0:ORIG] = \
                o[:, RECV * b:RECV * (b + 1)].T
    return out


def run(inputs, trace=False, use_f32r=True, **trace_kwargs):
    key = use_f32r
    if key not in _PROGRAM_CACHE:
        _PROGRAM_CACHE[key] = build_program(use_f32r=use_f32r)
    nc = _PROGRAM_CACHE[key]
    in_maps = _host_prep(
        inputs["z_aug"], inputs["eW0"], inputs["eb0"], inputs["eW1"],
        inputs["eb1"], inputs["eW2"], inputs["eb2"], inputs["uW0"],
        inputs["ub0"], inputs["uW1"], inputs["ub1"], inputs["uW2"],
        inputs["ub2"],
    )
    res = run_bass_kernel_spmd(
        nc, in_maps, list(range(NCORES)), trace=trace, **trace_kwargs
    )
    out = _assemble(res.results, np.asarray(inputs["z_aug"]).dtype)
    return out, res


def kernel(**inputs):
    out, _ = run(inputs, trace=False)
    return out


# revision 9
# speedup vs baseline: 1.3418x; 1.3418x over previous
"""Trainium2 Bass kernel for AugmentedGraphNeuralODEFunc.

Reference computation (B=4, N=512, AUG=32, ORIG=16, HID=128):
  edge_features[b,i,j] = [z_i(32), z_j(32), p_i-p_j(3), |p_i-p_j|(1),
                          ps_i-ps_j(3), |ps_i-ps_j|(1)]       (72)
  msg = MLP(72->128->128->16) per edge; agg_i = sum_j msg_ij
  d_evolving = MLP(32->128->128->16)([z_i[:16], agg_i]); static half -> 0

Algebraic restructure used on device:
  layer1 pre-act for receiver i, sender j:
    h1[:,j] = W_B^T z_j + A_i + dist_ij * v + dist_s_ij * w
  where A_i = W_A^T z_i + eb0 (diff terms fold into W_A/W_B since
  diff = p_i - p_j is linear in z), v/w are the dist rows of eW0.
  dist^2 via the Gram identity r_i + r_j - 2 p_i.p_j as ONE K=5 matmul,
  then clamp + sqrt.  Layer 3 + bias commute with the sum over j:
    agg_i = (sum_j relu(h2_ij)) @ eW2 + N*eb2.

Sharding: receivers (dim 1 of the NxN edge tensor) split across 8 cores,
64 receivers x 4 batches = 256 receiver-pairs per core; the sum over
senders is local so there is no cross-core communication.

Per (b,i) pair on device:
  DMA   : A_i row -> LZ[k] row 32 (rotating lhsT buffers)
  mm_z  : psum  = LZ^T @ ZT1[b]              K=33
  mm_vw : psum += VW128[32a:32a+2]^T @ PS[b][32a:32a+2, s]   K=2
  DVE   : h1 = relu(psum)                    (PSUM->SBUF, 2 pairs/op)
  mm2   : psum1 = eW1^T @ h1                 K=128
  ACT   : relu(psum1 + eb1) with accum_out -> S[:, pair]  (the j-sum)
Tail (256 pairs at once): agg = eW2^T S + N*eb2; update MLP 32->128->128->16.

Host-side prep is restricted to O(N)-sized layout/weight folding (zt
transpose, Gram operand rows, weight scatter-folds); all O(N^2) work and
all O(N*HID) matmuls run on device.
"""

import ml_dtypes
import numpy as np

import concourse.bass as bass
import concourse.tile as tile
from concourse import bacc, mybir
from concourse.bass_utils import run_bass_kernel_spmd

ORIG = 16
AUG = 32
HID = 128
B = 4
N = 512
NCORES = 8
RECV = N // NCORES          # 64 receivers per core
PAIRS = B * RECV            # 256 (b, i) pairs per core

F32 = mybir.dt.float32
F32R = mybir.dt.float32r
BF16 = mybir.dt.bfloat16
AluOp = mybir.AluOpType
Act = mybir.ActivationFunctionType

_PROGRAM_CACHE = {}


def build_program(use_f32r=True):
    nc = bacc.Bacc("TRN2", target_bir_lowering=False, debug=False)

    MF = BF16 if use_f32r else F32

    def din(name, shape, dt=F32):
        return nc.dram_tensor(name, shape, dt, kind="ExternalInput")

    zt1_d = din("zt1", [B, 33, N], MF)        # z[b].T (32 rows) + ones row
    zr_d = din("zr", [B, 33, RECV])       # receiver slice of zt1 (per-core)
    gl_d = din("gl", [B, 37, RECV])       # Gram lhsT rows (per-core)
    gr_d = din("gr", [B, 37, N])          # Gram rhs rows
    wb32_d = din("wb32", [32, HID])       # folded sender-side layer1 weights
    wa33_d = din("wa33", [33, HID])       # folded receiver-side weights + eb0
    vw128_d = din("vw128", [128, HID], MF)    # v at rows 32a, w at rows 32a+1
    ew1_d = din("ew1", [HID, HID], MF)
    eb1_d = din("eb1", [HID, 1])
    ew2_d = din("ew2", [HID, ORIG])
    nb2_d = din("nb2", [ORIG, 1])         # N * eb2
    uw0_d = din("uw0", [AUG, HID])
    ub0_d = din("ub0", [HID, 1])
    uw1_d = din("uw1", [HID, HID])
    ub1_d = din("ub1", [HID, 1])
    uw2_d = din("uw2", [HID, ORIG])
    ub2_d = din("ub2", [ORIG, 1])
    out_d = nc.dram_tensor("out", [ORIG, PAIRS], F32, kind="ExternalOutput")

    with tile.TileContext(nc) as tc:
        with (
            tc.tile_pool(name="const", bufs=1) as cp,
            tc.tile_pool(name="work", bufs=2) as wp,
            tc.tile_pool(name="ps0", bufs=2, space=bass.MemorySpace.PSUM) as pp0,
            tc.tile_pool(name="ps1", bufs=4, space=bass.MemorySpace.PSUM) as pp1,
        ):
            # ---------------- load constants / inputs ----------------
            ZT1 = [cp.tile([33, N], MF, name=f"zt1_{b}") for b in range(B)]
            ZR = [cp.tile([33, RECV], F32, name=f"zr_{b}") for b in range(B)]
            GL = [cp.tile([37, RECV], F32, name=f"gl_{b}") for b in range(B)]
            GR = [cp.tile([37, N], F32, name=f"gr_{b}") for b in range(B)]
            for b in range(B):
                nc.sync.dma_start(ZT1[b][:], zt1_d[b])
                nc.sync.dma_start(ZR[b][:], zr_d[b])
                nc.sync.dma_start(GL[b][:], gl_d[b])
                nc.sync.dma_start(GR[b][:], gr_d[b])

            WB32 = cp.tile([32, HID], F32, name="wb32")
            WA33 = cp.tile([33, HID], F32, name="wa33")
            VW128 = cp.tile([128, HID], MF, name="vw128")
            EW1 = cp.tile([HID, HID], MF, name="ew1")
            EB1 = cp.tile([HID, 1], F32, name="eb1")
            EW2 = cp.tile([HID, ORIG], F32, name="ew2")
            NB2 = cp.tile([ORIG, 1], F32, name="nb2")
            UW0 = cp.tile([AUG, HID], F32, name="uw0")
            UB0 = cp.tile([HID, 1], F32, name="ub0")
            UW1 = cp.tile([HID, HID], F32, name="uw1")
            UB1 = cp.tile([HID, 1], F32, name="ub1")
            UW2 = cp.tile([HID, ORIG], F32, name="uw2")
            UB2 = cp.tile([ORIG, 1], F32, name="ub2")
            for t, d in [
                (WB32, wb32_d), (WA33, wa33_d), (VW128, vw128_d), (EW1, ew1_d),
                (EB1, eb1_d), (EW2, ew2_d), (NB2, nb2_d), (UW0, uw0_d),
                (UB0, ub0_d), (UW1, uw1_d), (UB1, ub1_d), (UW2, uw2_d),
                (UB2, ub2_d),
            ]:
                nc.sync.dma_start(t[:], d[:])

            # lhsT buffers for mm_z: rows 0:32 = W_B (static), row 32 = A_i
            LZ = [cp.tile([33, HID], MF, name=f"lz_{k}") for k in range(4)]
            for k in range(4):
                nc.vector.tensor_copy(LZ[k][0:32, :], WB32[:])

            # ---------------- A rows per receiver --------------------
            # AER[b][p, :] = z_i^T W_A + eb0   (row-major, feeds LZ row 32)
            AER = [cp.tile([RECV, HID], MF, name=f"aer_{b}") for b in range(B)]
            for b in range(B):
                a_ps = pp1.tile([RECV, HID], F32, tag="psum1", name="a_ps")
                nc.tensor.matmul(a_ps[:], ZR[b][:], WA33[:], start=True, stop=True)
                nc.vector.tensor_copy(AER[b][:], a_ps[:])

            # ---------------- main loop over b -----------------------
            S = cp.tile([HID, PAIRS], F32, name="s_acc")

            for b in range(B):
                # distances: Gram -> clamp -> sqrt -> pair-staged layout
                # PS_b partition 64+32c+t, slot s, receiver p = 32c+s;
                # t=0 dist, t=1 dist_s.  Bases {64, 96} keep the K=2
                # matmul on PE row groups disjoint from mm_z (rows 0-63)
                # so it overlaps the neighboring mm_z on the array.
                PS_b = wp.tile([128, RECV // 2, N], MF, tag="pairstage",
                               name="pairstage")
                for half in range(2):
                    ro = 32 * half
                    g_ps = pp1.tile([RECV, N], F32, tag="psum1", name="g_ps")
                    nc.tensor.matmul(
                        g_ps[:], GL[b][ro:ro + 5, :], GR[b][ro:ro + 5, :],
                        start=True, stop=True,
                    )
                    d2 = wp.tile([RECV, N], F32, tag="d2", name="d2")
                    nc.vector.tensor_scalar(
                        out=d2[:], in0=g_ps[:],
                        scalar1=0.0, scalar2=None, op0=AluOp.max,
                    )
                    dsq = wp.tile([RECV, N], MF, tag="dsq", name="dsq")
                    nc.scalar.sqrt(dsq[:], d2[:])
                    # partition remap p=32c+s -> (64+32c+half, s) via 2
                    # contiguous-partition DMAs
                    for c in range(2):
                        bp = 64 + 32 * c + half
                        nc.sync.dma_start(
                            PS_b[bp:bp + 1, :, :],
                            dsq[32 * c:32 * c + 32, :],
                        )

                for pp_i in range(0, RECV, 2):
                    g = (RECV * b + pp_i) // 2
                    psum0 = pp0.tile([128, 2 * N], F32, tag="psum0",
                                     name="psum0")
                    for t in range(2):
                        p = pp_i + t
                        q = RECV * b + p
                        k = q % 4
                        c = p // 32
                        s = p % 32
                        bp = 64 + 32 * c
                        nc.sync.dma_start(
                            LZ[k][32:33, :], AER[b][p:p + 1, :]
                        )
                        # outer-product matmul first (start=True), on PE
                        # rows 64/96 so the two pairs' vw matmuls and the
                        # previous mm_z overlap on disjoint row groups
                        nc.tensor.matmul(
                            psum0[:, N * t:N * (t + 1)],
                            VW128[bp:bp + 2, :],
                            PS_b[bp:bp + 2, s, :],
                            start=True, stop=False,
                            tile_position=(bp, 0),
                        )
                    for t in range(2):
                        q = RECV * b + pp_i + t
                        k = q % 4
                        nc.tensor.matmul(
                            psum0[:, N * t:N * (t + 1)], LZ[k][:], ZT1[b][:],
                            start=False, stop=True,
                        )
                    h1 = wp.tile([128, 2 * N], MF, tag="h1", name="h1")
                    if g % 8 == 7:
                        # every 8th group's relu-extract runs on ACT to
                        # balance DVE/ACT PSUM-read load
                        nc.scalar.activation(
                            out=h1[:], in_=psum0[:], func=Act.Relu,
                        )
                    else:
                        nc.vector.tensor_scalar(
                            out=h1[:], in0=psum0[:],
                            scalar1=0.0, scalar2=None, op0=AluOp.max,
                        )
                    for t in range(2):
                        q = RECV * b + pp_i + t
                        psum1 = pp1.tile([HID, N], F32, tag="psum1",
                                         name="psum1")
                        nc.tensor.matmul(
                            psum1[:], EW1[:],
                            h1[:, N * t:N * (t + 1)],
                            start=True, stop=True,
                        )
                        # relu(h2 + eb1), summed over senders -> S[:, q]
                        nc.scalar.activation(
                            out=psum1[:], in_=psum1[:], func=Act.Relu,
                            bias=EB1[:], scale=1.0,
                            accum_out=S[:, q:q + 1],
                        )

            # ---------------- tail: agg + update MLP -----------------
            U = cp.tile([AUG, PAIRS], F32, name="u_in")
            for b in range(B):
                nc.vector.tensor_copy(
                    U[0:ORIG, RECV * b:RECV * (b + 1)], ZR[b][0:ORIG, :]
                )
            agg_ps = pp1.tile([ORIG, PAIRS], F32, tag="psum1", name="agg_ps")
            nc.tensor.matmul(agg_ps[:], EW2[:], S[:], start=True, stop=True)
            AGGSB = cp.tile([ORIG, PAIRS], F32, name="aggsb")
            nc.vector.tensor_scalar(
                out=AGGSB[:], in0=agg_ps[:],
                scalar1=NB2[:], scalar2=None, op0=AluOp.add,
            )
            nc.sync.dma_start(U[ORIG:AUG, :], AGGSB[:])

            u1_ps = pp1.tile([HID, PAIRS], F32, tag="psum1", name="u1_ps")
            nc.tensor.matmul(u1_ps[:], UW0[:], U[:], start=True, stop=True)
            HU1 = cp.tile([HID, PAIRS], F32, name="hu1")
            nc.scalar.activation(
                out=HU1[:], in_=u1_ps[:], func=Act.Relu, bias=UB0[:], scale=1.0
            )
            u2_ps = pp1.tile([HID, PAIRS], F32, tag="psum1", name="u2_ps")
            nc.tensor.matmul(u2_ps[:], UW1[:], HU1[:], start=True, stop=True)
            HU2 = cp.tile([HID, PAIRS], F32, name="hu2")
            nc.scalar.activation(
                out=HU2[:], in_=u2_ps[:], func=Act.Relu, bias=UB1[:], scale=1.0
            )
            u3_ps = pp1.tile([ORIG, PAIRS], F32, tag="psum1", name="u3_ps")
            nc.tensor.matmul(u3_ps[:], UW2[:], HU2[:], start=True, stop=True)
            OUTSB = cp.tile([ORIG, PAIRS], F32, name="outsb")
            nc.vector.tensor_scalar(
                out=OUTSB[:], in0=u3_ps[:],
                scalar1=UB2[:], scalar2=None, op0=AluOp.add,
            )
            nc.sync.dma_start(out_d[:], OUTSB[:])

    nc.compile()
    return nc


def _host_prep(z_aug, eW0, eb0, eW1, eb1, eW2, eb2,
               uW0, ub0, uW1, ub1, uW2, ub2):
    f = np.float32
    z = np.ascontiguousarray(z_aug, dtype=f)
    zt = z.transpose(0, 2, 1)                            # [B, 32, N]
    zt1 = np.concatenate([zt, np.ones((B, 1, N), f)], axis=1)  # [B, 33, N]

    # Gram rhs rows (same for all cores):
    # rows 0-2 p_j, 3 ones, 4 r_j;  rows 32-34 ps_j, 35 ones, 36 rs_j
    gr = np.zeros((B, 37, N), f)
    p = zt[:, 0:3, :]
    ps = zt[:, 16:19, :]
    r = (p * p).sum(axis=1)
    rs = (ps * ps).sum(axis=1)
    gr[:, 0:3] = p
    gr[:, 3] = 1.0
    gr[:, 4] = r
    gr[:, 32:35] = ps
    gr[:, 35] = 1.0
    gr[:, 36] = rs

    eW0 = np.asarray(eW0, f)
    WA = eW0[0:32].copy()
    WA[0:3] += eW0[64:67]
    WA[16:19] += eW0[68:71]
    wa33 = np.concatenate([WA, np.asarray(eb0, f)[None, :]], axis=0)
    WB = eW0[32:64].copy()
    WB[0:3] -= eW0[64:67]
    WB[16:19] -= eW0[68:71]
    vw128 = np.zeros((128, HID), f)
    vw128[[64, 96], :] = eW0[67]
    vw128[[65, 97], :] = eW0[71]

    common = {
        "zt1": np.ascontiguousarray(zt1).astype(ml_dtypes.bfloat16),
        "gr": gr,
        "wb32": np.ascontiguousarray(WB),
        "wa33": np.ascontiguousarray(wa33),
        "vw128": vw128.astype(ml_dtypes.bfloat16),
        "ew1": np.ascontiguousarray(np.asarray(eW1, f)).astype(ml_dtypes.bfloat16),
        "eb1": np.asarray(eb1, f).reshape(HID, 1).copy(),
        "ew2": np.ascontiguousarray(np.asarray(eW2, f)),
        "nb2": (np.asarray(eb2, f) * np.float32(N)).reshape(ORIG, 1).copy(),
        "uw0": np.ascontiguousarray(np.asarray(uW0, f)),
        "ub0": np.asarray(ub0, f).reshape(HID, 1).copy(),
        "uw1": np.ascontiguousarray(np.asarray(uW1, f)),
        "ub1": np.asarray(ub1, f).reshape(HID, 1).copy(),
        "uw2": np.ascontiguousarray(np.asarray(uW2, f)),
        "ub2": np.asarray(ub2, f).reshape(ORIG, 1).copy(),
    }
    in_maps = []
    for c in range(NCORES):
        sl = slice(RECV * c, RECV * (c + 1))
        m = dict(common)
        m["zr"] = np.ascontiguousarray(zt1[:, :, sl])
        gl = np.zeros((B, 37, RECV), f)
        gl[:, 0:3] = -2.0 * p[:, :, sl]
        gl[:, 3] = r[:, sl]
        gl[:, 4] = 1.0
        gl[:, 32:35] = -2.0 * ps[:, :, sl]
        gl[:, 35] = rs[:, sl]
        gl[:, 36] = 1.0
        m["gl"] = gl
        in_maps.append(m)
    return in_maps


def _assemble(results, dtype):
    out = np.zeros((B, N, AUG), dtype=dtype)
    for c in range(NCORES):
        o = results[c]["out"]                 # [ORIG, PAIRS]
        for b in range(B):
            out[b, RECV * c:RECV * (c + 1), 0:ORIG] = \
                o[:, RECV * b:RECV * (b + 1)].T
    return out


def run(inputs, trace=False, use_f32r=True, **trace_kwargs):
    key = use_f32r
    if key not in _PROGRAM_CACHE:
        _PROGRAM_CACHE[key] = build_program(use_f32r=use_f32r)
    nc = _PROGRAM_CACHE[key]
    in_maps = _host_prep(
        inputs["z_aug"], inputs["eW0"], inputs["eb0"], inputs["eW1"],
        inputs["eb1"], inputs["eW2"], inputs["eb2"], inputs["uW0"],
        inputs["ub0"], inputs["uW1"], inputs["ub1"], inputs["uW2"],
        inputs["ub2"],
    )
    res = run_bass_kernel_spmd(
        nc, in_maps, list(range(NCORES)), trace=trace, **trace_kwargs
    )
    out = _assemble(res.results, np.asarray(inputs["z_aug"]).dtype)
    return out, res


def kernel(**inputs):
    out, _ = run(inputs, trace=False)
    return out
